# revision 29
# baseline (speedup 1.0000x reference)
"""DropBlock1D + Linear classifier forward, data-parallel over 8 trn2 cores.

Computes out = (x * keep) @ W.T + b where keep zeroes a contiguous
BLOCK_LEN-channel window per row, starting at starts[row].

Strategy:
  - Host: sort rows by start value, shard sorted rows across 8 cores.
    Within each 128-row tile the starts then span a narrow range, so the
    per-row zero-window for the whole tile lives inside one WIN-wide
    column window (WIN ~ 896 instead of 4096).  Masking cost drops ~4.5x.
  - Device (per core, fp32 throughout):
      load x tile [128, 4096] (natural layout)
      mask the WIN-wide window in place on VectorE:
          tmp  = |iota - (start + 409)|      (tensor_scalar, 2x mode)
          keep = tmp > 409                    (tensor_scalar, 2x mode)
          x[:, w:w+WIN] *= keep               (tensor_tensor)
      transpose 128x128 chunks on TensorE (identity matmul) -> PSUM,
      copy PSUM->SBUF on ScalarE/VectorE, then accumulate
          psum_out[64, 512] += Wt_chunk.T @ xT_chunk   over 32 chunks
      add bias during PSUM->SBUF copy, transpose [64,512] -> [512,64],
      DMA out.
  - Host: gather per-core outputs, undo the sort permutation.
"""

import os
import sys

sys.path.insert(0, "/opt/trn_rl_repo")

import numpy as np

B, C, NCLS = 16384, 4096, 64
L = 819
H = (L - 1) // 2  # 409
NCORES = 8
RPC = B // NCORES  # 2048 rows per core
P = 128
KCH = C // P  # 32 contraction chunks

_nc_cache = {}
last_exec_ns = None
last_results = None


def _build(rpc, win, super_):
    """Build the per-core Bass kernel. rpc rows per core, window width win,
    super_ 128-row tiles per supertile (product free dim = 128*super_)."""
    import concourse.bass as bass
    import concourse.mybir as mybir
    import concourse.tile as tile
    from concourse import bacc
    from concourse.masks import make_identity

    f32 = mybir.dt.float32
    i32 = mybir.dt.int32
    Alu = mybir.AluOpType

    tiles_pc = rpc // P
    nsup = tiles_pc // super_
    nfree = P * super_  # product matmul free dim

    nc = bacc.Bacc("TRN2", target_bir_lowering=False, debug=False,
                   enable_asserts=False)

    xs2_d = nc.dram_tensor("xs2", [rpc, 2 * C], f32, kind="ExternalInput")
    wt_d = nc.dram_tensor("wt", [P, KCH, NCLS], f32, kind="ExternalInput")
    bcol_d = nc.dram_tensor("bcol", [NCLS, 1], f32, kind="ExternalInput")
    iota_d = nc.dram_tensor("iota", [P, win], f32, kind="ExternalInput")
    nctr_d = nc.dram_tensor("nctr", [P, tiles_pc], f32, kind="ExternalInput")
    woff_d = nc.dram_tensor("woff", [1, 1], i32, kind="ExternalInput")
    out_d = nc.dram_tensor("out", [rpc, NCLS], f32, kind="ExternalOutput")

    with tile.TileContext(nc) as tc:
        with (
            tc.tile_pool(name="const", bufs=1) as constp,
            tc.tile_pool(name="xsb", bufs=2) as xp,
            tc.tile_pool(name="xt", bufs=3) as xtp,
            tc.tile_pool(name="msk", bufs=1) as mp,
            tc.tile_pool(name="ostage", bufs=2) as op_,
            tc.tile_pool(name="psxt", bufs=2, space="PSUM") as pxtp,
            tc.tile_pool(name="psout", bufs=2, space="PSUM") as potp,
            tc.tile_pool(name="psotr", bufs=2, space="PSUM") as ptrp,
        ):
            wt_sb = constp.tile([P, KCH, NCLS], f32)
            nc.sync.dma_start(wt_sb, wt_d.ap())
            iota_sb = constp.tile([P, win], f32)
            nc.sync.dma_start(iota_sb, iota_d.ap())
            nctr_sb = constp.tile([P, tiles_pc], f32)
            nc.sync.dma_start(nctr_sb, nctr_d.ap())
            woff_sb = constp.tile([1, 1], i32)
            nc.sync.dma_start(woff_sb, woff_d.ap())
            bcol_sb = constp.tile([NCLS, 1], f32)
            nc.sync.dma_start(bcol_sb, bcol_d.ap())
            ident = constp.tile([P, P], f32)
            make_identity(nc, ident)

            # per-core channel-rotation offset for the X loads (the host
            # rotates W identically, so the contraction pairing is preserved)
            wv = nc.values_load(
                woff_sb[0:1, 0:1], min_val=0, max_val=C,
                skip_runtime_bounds_check=True,
                engines=(mybir.EngineType.SP,),
            )

            for s in range(nsup):
                X = xp.tile([P, super_, C], f32, tag="X")
                nc.sync.dma_start(
                    X,
                    xs2_d.ap()[s * nfree:(s + 1) * nfree, :]
                    .rearrange("(j p) c -> p j c", p=P)[:, :, bass.ds(wv, C)],
                )
                keep4 = mp.tile([P, super_, win], f32, tag="keep")
                for j in range(super_):
                    t = s * super_ + j
                    tmp = mp.tile([P, win], f32, tag="tmp")
                    # tmp = (iota_local - (start - w_core + H))^2 on ScalarE
                    # (walrus rejects abs_max as a tensor_scalar op1)
                    nc.scalar.activation(
                        tmp, iota_sb, mybir.ActivationFunctionType.Square,
                        bias=nctr_sb[:, t:t + 1], scale=1.0,
                    )
                    # keep = dist^2 > (H+0.5)^2  (exact: distances are ints)
                    nc.vector.tensor_scalar(
                        keep4[:, j, :], tmp, (H + 0.5) ** 2, None, Alu.is_gt
                    )
                # masked multiply on the (static) leading window columns
                nc.vector.tensor_tensor(
                    X[:, :, 0:win], X[:, :, 0:win], keep4, Alu.mult,
                )

                po = potp.tile([NCLS, nfree], f32, tag="po")
                for k in range(KCH):
                    px = pxtp.tile([P, nfree], f32, tag="px")
                    for j in range(super_):
                        nc.tensor.transpose(
                            px[:, j * P:(j + 1) * P],
                            X[:, j, k * P:(k + 1) * P],
                            ident,
                        )
                    xt_t = xtp.tile([P, nfree], f32, tag="xt")
                    if k % 8 < 3:
                        nc.vector.tensor_copy(xt_t, px)
                    else:
                        nc.scalar.copy(xt_t, px)
                    nc.tensor.matmul(
                        po, wt_sb[:, k, :], xt_t,
                        start=(k == 0), stop=(k == KCH - 1),
                    )

                osb = op_.tile([NCLS, nfree], f32, tag="osb")
                nc.vector.tensor_tensor(
                    osb, po, bcol_sb[:, 0:1].to_broadcast((NCLS, nfree)),
                    Alu.add,
                )
                ost = op_.tile([P, super_, NCLS], f32, tag="ost")
                for j in range(super_):
                    pr = ptrp.tile([P, NCLS], f32, tag="pr")
                    nc.tensor.transpose(
                        pr, osb[:, j * P:(j + 1) * P], ident[0:NCLS, 0:NCLS]
                    )
                    nc.vector.tensor_copy(ost[:, j, :], pr)
                nc.sync.dma_start(
                    out_d.ap()[s * nfree:(s + 1) * nfree, :]
                    .rearrange("(j p) n -> p j n", p=P),
                    ost,
                )

    nc.compile()
    return nc


def _get_nc(win, rpc=RPC, super_=4):
    key = (rpc, win, super_)
    if key not in _nc_cache:
        _nc_cache[key] = _build(rpc, win, super_)
    return _nc_cache[key]


def _install_ntff_shim():
    """Provide antenv.axon_hooks (absent in this container) so
    run_bass_kernel_spmd(trace=True) can drive NTFF profiling via the
    axon PJRT .so. Only used for local perf measurement (KERNEL_TRACE=1)."""
    import contextlib
    import ctypes
    import sys as _sys
    import types

    try:
        from antenv.axon_hooks import get_axon_ntff_profile_hook  # noqa: F401
        return
    except ImportError:
        pass

    so_path = "/opt/axon/libaxon_pjrt.so"
    lib = ctypes.CDLL(so_path)
    if not hasattr(lib, "axon_start_nrt_profile"):
        return
    lib.axon_start_nrt_profile.argtypes = [
        ctypes.POINTER(ctypes.c_int64), ctypes.c_size_t,
    ]
    lib.axon_start_nrt_profile.restype = ctypes.c_int64
    lib.axon_stop_nrt_profile.argtypes = [ctypes.c_char_p]
    lib.axon_stop_nrt_profile.restype = ctypes.c_int64

    @contextlib.contextmanager
    def _hook(output_dir, device_ids):
        import jax
        jax.devices()
        if device_ids:
            ids = (ctypes.c_int64 * len(device_ids))(*device_ids)
            rc = lib.axon_start_nrt_profile(ids, len(device_ids))
        else:
            rc = lib.axon_start_nrt_profile(None, 0)
        if rc != 0:
            raise RuntimeError(f"axon_start_nrt_profile rc={rc}")
        try:
            yield
        finally:
            n = lib.axon_stop_nrt_profile(str(output_dir).encode())
            print(f"ntff profile: {n} file(s) -> {output_dir}", file=_sys.stderr)

    mod = types.ModuleType("antenv.axon_hooks")
    mod.get_axon_ntff_profile_hook = lambda: _hook
    mod.set_axon_ntff_profile_hook = lambda h: None
    _sys.modules["antenv.axon_hooks"] = mod


def kernel(x, W, b, starts):
    global last_exec_ns, last_results
    from concourse.bass_utils import run_bass_kernel_spmd

    x = np.ascontiguousarray(np.asarray(x, dtype=np.float32))
    W32 = np.asarray(W, dtype=np.float32)
    b32 = np.asarray(b, dtype=np.float32)
    st = np.asarray(starts, dtype=np.int32).reshape(-1)

    tiles_pc = RPC // P

    perm = np.argsort(st, kind="stable")
    st_s = st[perm]

    # per-core channel rotation: core m handles sorted rows [m*RPC,(m+1)*RPC),
    # whose starts span a narrow range; rotate channels by w_m = min(start) so
    # every row's zero-block lands in the leading `win` rotated columns
    st_c = st_s.reshape(NCORES, RPC)
    w = st_c.min(axis=1).astype(np.int32)          # [NCORES]
    span = (st_c.max(axis=1) - w).astype(np.int64)
    win0 = int(span.max()) + L
    win = min(C, ((win0 + 127) // 128) * 128)
    assert (st_c.max(axis=1) + L <= w + win).all()

    nctr = -(st_s - np.repeat(w, RPC) + H).astype(np.float32)
    WtT = np.ascontiguousarray(W32.T)              # [C, NCLS]
    iota = np.ascontiguousarray(
        np.broadcast_to(np.arange(win, dtype=np.float32), (P, win))
    )
    bcol = np.ascontiguousarray(b32.reshape(NCLS, 1))

    nc = _get_nc(win)

    in_maps = []
    for m in range(NCORES):
        rows = perm[m * RPC:(m + 1) * RPC]
        xs = x[rows]
        wm = int(w[m])
        # rotated weights: rotated channel c <-> original channel (wm+c)%C
        wrot = np.take(WtT, (wm + np.arange(C)) % C, axis=0)
        in_maps.append({
            "xs2": np.concatenate([xs, xs], axis=1),
            "wt": np.ascontiguousarray(
                wrot.reshape(KCH, P, NCLS).transpose(1, 0, 2)
            ),
            "bcol": bcol,
            "iota": iota,
            "nctr": np.ascontiguousarray(
                nctr[m * RPC:(m + 1) * RPC].reshape(tiles_pc, P).T
            ),
            "woff": np.array([[wm]], dtype=np.int32),
        })

    trace = bool(int(os.environ.get("KERNEL_TRACE", "0")))
    if trace:
        _install_ntff_shim()
    res = run_bass_kernel_spmd(
        nc, in_maps, core_ids=list(range(NCORES)), trace=trace
    )
    last_exec_ns = res.exec_time_ns
    last_results = res

    out_s = np.concatenate([r["out"] for r in res.results], axis=0)
    out = np.empty((B, NCLS), dtype=np.float32)
    out[perm] = out_s
    return out


# revision 35
# speedup vs baseline: 1.5352x; 1.5352x over previous
"""DropBlock1D + Linear classifier forward, data-parallel over 8 trn2 cores.

Computes out = (x * keep) @ W.T + b where keep zeroes a contiguous
BLOCK_LEN-channel window per row, starting at starts[row].

Strategy:
  - Host: sort rows by start value, shard sorted rows across 8 cores.
    Within each 128-row tile the starts then span a narrow range, so the
    per-row zero-window for the whole tile lives inside one WIN-wide
    column window (WIN ~ 896 instead of 4096).  Masking cost drops ~4.5x.
  - Device (per core, fp32 throughout):
      load x tile [128, 4096] (natural layout)
      mask the WIN-wide window in place on VectorE:
          tmp  = |iota - (start + 409)|      (tensor_scalar, 2x mode)
          keep = tmp > 409                    (tensor_scalar, 2x mode)
          x[:, w:w+WIN] *= keep               (tensor_tensor)
      transpose 128x128 chunks on TensorE (identity matmul) -> PSUM,
      copy PSUM->SBUF on ScalarE/VectorE, then accumulate
          psum_out[64, 512] += Wt_chunk.T @ xT_chunk   over 32 chunks
      add bias during PSUM->SBUF copy, transpose [64,512] -> [512,64],
      DMA out.
  - Host: gather per-core outputs, undo the sort permutation.
"""

import os
import sys

sys.path.insert(0, "/opt/trn_rl_repo")

import numpy as np

B, C, NCLS = 16384, 4096, 64
L = 819
H = (L - 1) // 2  # 409
NCORES = 8
RPC = B // NCORES  # 2048 rows per core
P = 128
KCH = C // P  # 32 contraction chunks

_nc_cache = {}
last_exec_ns = None
last_results = None


def _build(rpc, win, super_, f32r_mm=True, f32r_tr=True):
    """Build the per-core Bass kernel. rpc rows per core, window width win,
    super_ 128-row tiles per supertile (product free dim = 128*super_).
    f32r_mm/f32r_tr: run product matmuls / transposes as float32r (full-rate
    PE streaming instead of fp32's 4-cycles-per-row double-pass)."""
    import concourse.bass as bass
    import concourse.mybir as mybir
    import concourse.tile as tile
    from concourse import bacc
    from concourse.masks import make_identity

    f32 = mybir.dt.float32
    f32r = mybir.dt.float32r
    i32 = mybir.dt.int32
    Alu = mybir.AluOpType
    mm_dt = f32r if f32r_mm else f32
    tr_dt = f32r if f32r_tr else f32

    tiles_pc = rpc // P
    nsup = tiles_pc // super_
    nfree = P * super_  # product matmul free dim

    nc = bacc.Bacc("TRN2", target_bir_lowering=False, debug=False,
                   enable_asserts=False)

    xs2_d = nc.dram_tensor("xs2", [rpc, 2 * C], f32, kind="ExternalInput")
    wt_d = nc.dram_tensor("wt", [P, KCH, NCLS], mm_dt, kind="ExternalInput")
    bcol_d = nc.dram_tensor("bcol", [NCLS, 1], f32, kind="ExternalInput")
    iota_d = nc.dram_tensor("iota", [P, win], f32, kind="ExternalInput")
    nctr_d = nc.dram_tensor("nctr", [P, tiles_pc], f32, kind="ExternalInput")
    woff_d = nc.dram_tensor("woff", [1, 1], i32, kind="ExternalInput")
    out_d = nc.dram_tensor("out", [rpc, NCLS], f32, kind="ExternalOutput")

    with tile.TileContext(nc) as tc:
        with (
            tc.tile_pool(name="const", bufs=1) as constp,
            tc.tile_pool(name="xsb", bufs=2) as xp,
            tc.tile_pool(name="xt", bufs=3) as xtp,
            tc.tile_pool(name="msk", bufs=1) as mp,
            tc.tile_pool(name="ostage", bufs=2) as op_,
            tc.tile_pool(name="psxt", bufs=2, space="PSUM") as pxtp,
            tc.tile_pool(name="psout", bufs=2, space="PSUM") as potp,
            tc.tile_pool(name="psotr", bufs=2, space="PSUM") as ptrp,
        ):
            wt_sb = constp.tile([P, KCH, NCLS], mm_dt)
            nc.sync.dma_start(wt_sb, wt_d.ap())
            iota_sb = constp.tile([P, win], f32)
            nc.sync.dma_start(iota_sb, iota_d.ap())
            nctr_sb = constp.tile([P, tiles_pc], f32)
            nc.sync.dma_start(nctr_sb, nctr_d.ap())
            woff_sb = constp.tile([1, 1], i32)
            nc.sync.dma_start(woff_sb, woff_d.ap())
            bcol_sb = constp.tile([NCLS, 1], f32)
            nc.sync.dma_start(bcol_sb, bcol_d.ap())
            ident = constp.tile([P, P], f32)
            make_identity(nc, ident)

            # per-core channel-rotation offset for the X loads (the host
            # rotates W identically, so the contraction pairing is preserved)
            wv = nc.values_load(
                woff_sb[0:1, 0:1], min_val=0, max_val=C,
                skip_runtime_bounds_check=True,
                engines=(mybir.EngineType.SP,),
            )

            for s in range(nsup):
                X = xp.tile([P, super_, C], f32, tag="X")
                nc.sync.dma_start(
                    X,
                    xs2_d.ap()[s * nfree:(s + 1) * nfree, :]
                    .rearrange("(j p) c -> p j c", p=P)[:, :, bass.ds(wv, C)],
                )
                keep4 = mp.tile([P, super_, win], f32, tag="keep")
                for j in range(super_):
                    t = s * super_ + j
                    tmp = mp.tile([P, win], f32, tag="tmp")
                    # tmp = (iota_local - (start - w_core + H))^2 on ScalarE
                    # (walrus rejects abs_max as a tensor_scalar op1)
                    nc.scalar.activation(
                        tmp, iota_sb, mybir.ActivationFunctionType.Square,
                        bias=nctr_sb[:, t:t + 1], scale=1.0,
                    )
                    # keep = dist^2 > (H+0.5)^2  (exact: distances are ints)
                    nc.vector.tensor_scalar(
                        keep4[:, j, :], tmp, (H + 0.5) ** 2, None, Alu.is_gt
                    )
                # masked multiply on the (static) leading window columns
                nc.vector.tensor_tensor(
                    X[:, :, 0:win], X[:, :, 0:win], keep4, Alu.mult,
                )

                po = potp.tile([NCLS, nfree], f32, tag="po")
                for k in range(KCH):
                    px = pxtp.tile([P, nfree], f32, tag="px")
                    for j in range(super_):
                        nc.tensor.transpose(
                            px[:, j * P:(j + 1) * P],
                            X[:, j, k * P:(k + 1) * P],
                            ident,
                        )
                    xt_t = xtp.tile([P, nfree], mm_dt, tag="xt")
                    if k % 8 < 3:
                        nc.vector.tensor_copy(xt_t, px)
                    else:
                        nc.scalar.copy(xt_t, px)
                    nc.tensor.matmul(
                        po, wt_sb[:, k, :], xt_t,
                        start=(k == 0), stop=(k == KCH - 1),
                    )

                osb = op_.tile([NCLS, nfree], f32, tag="osb")
                nc.vector.tensor_tensor(
                    osb, po, bcol_sb[:, 0:1].to_broadcast((NCLS, nfree)),
                    Alu.add,
                )
                ost = op_.tile([P, super_, NCLS], f32, tag="ost")
                for j in range(super_):
                    pr = ptrp.tile([P, NCLS], f32, tag="pr")
                    nc.tensor.transpose(
                        pr, osb[:, j * P:(j + 1) * P], ident[0:NCLS, 0:NCLS]
                    )
                    nc.vector.tensor_copy(ost[:, j, :], pr)
                nc.sync.dma_start(
                    out_d.ap()[s * nfree:(s + 1) * nfree, :]
                    .rearrange("(j p) n -> p j n", p=P),
                    ost,
                )

    nc.compile()
    return nc


def _get_nc(win, rpc=RPC, super_=4):
    key = (rpc, win, super_)
    if key not in _nc_cache:
        _nc_cache[key] = _build(rpc, win, super_)
    return _nc_cache[key]


def _install_ntff_shim():
    """Provide antenv.axon_hooks (absent in this container) so
    run_bass_kernel_spmd(trace=True) can drive NTFF profiling via the
    axon PJRT .so. Only used for local perf measurement (KERNEL_TRACE=1)."""
    import contextlib
    import ctypes
    import sys as _sys
    import types

    try:
        from antenv.axon_hooks import get_axon_ntff_profile_hook  # noqa: F401
        return
    except ImportError:
        pass

    so_path = "/opt/axon/libaxon_pjrt.so"
    lib = ctypes.CDLL(so_path)
    if not hasattr(lib, "axon_start_nrt_profile"):
        return
    lib.axon_start_nrt_profile.argtypes = [
        ctypes.POINTER(ctypes.c_int64), ctypes.c_size_t,
    ]
    lib.axon_start_nrt_profile.restype = ctypes.c_int64
    lib.axon_stop_nrt_profile.argtypes = [ctypes.c_char_p]
    lib.axon_stop_nrt_profile.restype = ctypes.c_int64

    @contextlib.contextmanager
    def _hook(output_dir, device_ids):
        import jax
        jax.devices()
        if device_ids:
            ids = (ctypes.c_int64 * len(device_ids))(*device_ids)
            rc = lib.axon_start_nrt_profile(ids, len(device_ids))
        else:
            rc = lib.axon_start_nrt_profile(None, 0)
        if rc != 0:
            raise RuntimeError(f"axon_start_nrt_profile rc={rc}")
        try:
            yield
        finally:
            n = lib.axon_stop_nrt_profile(str(output_dir).encode())
            print(f"ntff profile: {n} file(s) -> {output_dir}", file=_sys.stderr)

    mod = types.ModuleType("antenv.axon_hooks")
    mod.get_axon_ntff_profile_hook = lambda: _hook
    mod.set_axon_ntff_profile_hook = lambda h: None
    _sys.modules["antenv.axon_hooks"] = mod


def kernel(x, W, b, starts):
    global last_exec_ns, last_results
    from concourse.bass_utils import run_bass_kernel_spmd

    x = np.ascontiguousarray(np.asarray(x, dtype=np.float32))
    W32 = np.asarray(W, dtype=np.float32)
    b32 = np.asarray(b, dtype=np.float32)
    st = np.asarray(starts, dtype=np.int32).reshape(-1)

    tiles_pc = RPC // P

    perm = np.argsort(st, kind="stable")
    st_s = st[perm]

    # per-core channel rotation: core m handles sorted rows [m*RPC,(m+1)*RPC),
    # whose starts span a narrow range; rotate channels by w_m = min(start) so
    # every row's zero-block lands in the leading `win` rotated columns
    st_c = st_s.reshape(NCORES, RPC)
    w = st_c.min(axis=1).astype(np.int32)          # [NCORES]
    span = (st_c.max(axis=1) - w).astype(np.int64)
    win0 = int(span.max()) + L
    win = min(C, ((win0 + 127) // 128) * 128)
    assert (st_c.max(axis=1) + L <= w + win).all()

    nctr = -(st_s - np.repeat(w, RPC) + H).astype(np.float32)
    WtT = np.ascontiguousarray(W32.T)              # [C, NCLS]
    iota = np.ascontiguousarray(
        np.broadcast_to(np.arange(win, dtype=np.float32), (P, win))
    )
    bcol = np.ascontiguousarray(b32.reshape(NCLS, 1))

    nc = _get_nc(win)

    in_maps = []
    for m in range(NCORES):
        rows = perm[m * RPC:(m + 1) * RPC]
        xs = x[rows]
        wm = int(w[m])
        # rotated weights: rotated channel c <-> original channel (wm+c)%C
        wrot = np.take(WtT, (wm + np.arange(C)) % C, axis=0)
        in_maps.append({
            "xs2": np.concatenate([xs, xs], axis=1),
            "wt": np.ascontiguousarray(
                wrot.reshape(KCH, P, NCLS).transpose(1, 0, 2)
            ),
            "bcol": bcol,
            "iota": iota,
            "nctr": np.ascontiguousarray(
                nctr[m * RPC:(m + 1) * RPC].reshape(tiles_pc, P).T
            ),
            "woff": np.array([[wm]], dtype=np.int32),
        })

    trace = bool(int(os.environ.get("KERNEL_TRACE", "0")))
    if trace:
        _install_ntff_shim()
    res = run_bass_kernel_spmd(
        nc, in_maps, core_ids=list(range(NCORES)), trace=trace
    )
    last_exec_ns = res.exec_time_ns
    last_results = res

    out_s = np.concatenate([r["out"] for r in res.results], axis=0)
    out = np.empty((B, NCLS), dtype=np.float32)
    out[perm] = out_s
    return out


# revision 36
# speedup vs baseline: 1.7651x; 1.1498x over previous
"""DropBlock1D + Linear classifier forward, data-parallel over 8 trn2 cores.

Computes out = (x * keep) @ W.T + b where keep zeroes a contiguous
BLOCK_LEN-channel window per row, starting at starts[row].

Strategy:
  - Host: sort rows by start value, shard sorted rows across 8 cores.
    Within each 128-row tile the starts then span a narrow range, so the
    per-row zero-window for the whole tile lives inside one WIN-wide
    column window (WIN ~ 896 instead of 4096).  Masking cost drops ~4.5x.
  - Device (per core, fp32 throughout):
      load x tile [128, 4096] (natural layout)
      mask the WIN-wide window in place on VectorE:
          tmp  = |iota - (start + 409)|      (tensor_scalar, 2x mode)
          keep = tmp > 409                    (tensor_scalar, 2x mode)
          x[:, w:w+WIN] *= keep               (tensor_tensor)
      transpose 128x128 chunks on TensorE (identity matmul) -> PSUM,
      copy PSUM->SBUF on ScalarE/VectorE, then accumulate
          psum_out[64, 512] += Wt_chunk.T @ xT_chunk   over 32 chunks
      add bias during PSUM->SBUF copy, transpose [64,512] -> [512,64],
      DMA out.
  - Host: gather per-core outputs, undo the sort permutation.
"""

import os
import sys

sys.path.insert(0, "/opt/trn_rl_repo")

import numpy as np

B, C, NCLS = 16384, 4096, 64
L = 819
H = (L - 1) // 2  # 409
NCORES = 8
RPC = B // NCORES  # 2048 rows per core
P = 128
KCH = C // P  # 32 contraction chunks

_nc_cache = {}
last_exec_ns = None
last_results = None


def _build(rpc, win, super_, f32r_mm=True, f32r_tr=True):
    """Build the per-core Bass kernel. rpc rows per core, window width win,
    super_ 128-row tiles per supertile (product free dim = 128*super_).
    f32r_mm/f32r_tr: run product matmuls / transposes as float32r (full-rate
    PE streaming instead of fp32's 4-cycles-per-row double-pass)."""
    import concourse.bass as bass
    import concourse.mybir as mybir
    import concourse.tile as tile
    from concourse import bacc
    from concourse.masks import make_identity

    f32 = mybir.dt.float32
    f32r = mybir.dt.float32r
    i32 = mybir.dt.int32
    Alu = mybir.AluOpType
    mm_dt = f32r if f32r_mm else f32
    tr_dt = f32r if f32r_tr else f32

    tiles_pc = rpc // P
    nsup = tiles_pc // super_
    nfree = P * super_  # product matmul free dim

    nc = bacc.Bacc("TRN2", target_bir_lowering=False, debug=False,
                   enable_asserts=False)

    xs2_d = nc.dram_tensor("xs2", [rpc, 2 * C], tr_dt, kind="ExternalInput")
    wt_d = nc.dram_tensor("wt", [P, KCH, NCLS], mm_dt, kind="ExternalInput")
    bcol_d = nc.dram_tensor("bcol", [NCLS, 1], f32, kind="ExternalInput")
    iota_d = nc.dram_tensor("iota", [P, win], f32, kind="ExternalInput")
    nctr_d = nc.dram_tensor("nctr", [P, tiles_pc], f32, kind="ExternalInput")
    woff_d = nc.dram_tensor("woff", [1, 1], i32, kind="ExternalInput")
    out_d = nc.dram_tensor("out", [rpc, NCLS], f32, kind="ExternalOutput")

    with tile.TileContext(nc) as tc:
        with (
            tc.tile_pool(name="const", bufs=1) as constp,
            tc.tile_pool(name="xsb", bufs=2) as xp,
            tc.tile_pool(name="xt", bufs=4) as xtp,
            tc.tile_pool(name="msk", bufs=1) as mp,
            tc.tile_pool(name="ostage", bufs=2) as op_,
            tc.tile_pool(name="psxt", bufs=3, space="PSUM") as pxtp,
            tc.tile_pool(name="psout", bufs=2, space="PSUM") as potp,
            tc.tile_pool(name="psotr", bufs=2, space="PSUM") as ptrp,
        ):
            wt_sb = constp.tile([P, KCH, NCLS], mm_dt)
            nc.sync.dma_start(wt_sb, wt_d.ap())
            iota_sb = constp.tile([P, win], f32)
            nc.sync.dma_start(iota_sb, iota_d.ap())
            nctr_sb = constp.tile([P, tiles_pc], f32)
            nc.sync.dma_start(nctr_sb, nctr_d.ap())
            woff_sb = constp.tile([1, 1], i32)
            nc.sync.dma_start(woff_sb, woff_d.ap())
            bcol_sb = constp.tile([NCLS, 1], f32)
            nc.sync.dma_start(bcol_sb, bcol_d.ap())
            ident = constp.tile([P, P], f32)
            make_identity(nc, ident)
            identr = constp.tile([P, P], tr_dt)
            nc.vector.tensor_copy(identr, ident)

            # per-core channel-rotation offset for the X loads (the host
            # rotates W identically, so the contraction pairing is preserved)
            wv = nc.values_load(
                woff_sb[0:1, 0:1], min_val=0, max_val=C,
                skip_runtime_bounds_check=True,
                engines=(mybir.EngineType.SP,),
            )

            for s in range(nsup):
                X = xp.tile([P, super_, C], tr_dt, tag="X")
                nc.sync.dma_start(
                    X,
                    xs2_d.ap()[s * nfree:(s + 1) * nfree, :]
                    .rearrange("(j p) c -> p j c", p=P)[:, :, bass.ds(wv, C)],
                )
                keep4 = mp.tile([P, super_, win], f32, tag="keep")
                for j in range(super_):
                    t = s * super_ + j
                    tmp = mp.tile([P, win], f32, tag="tmp")
                    # tmp = (iota_local - (start - w_core + H))^2 on ScalarE
                    # (walrus rejects abs_max as a tensor_scalar op1)
                    nc.scalar.activation(
                        tmp, iota_sb, mybir.ActivationFunctionType.Square,
                        bias=nctr_sb[:, t:t + 1], scale=1.0,
                    )
                    # keep = dist^2 > (H+0.5)^2  (exact: distances are ints)
                    nc.vector.tensor_scalar(
                        keep4[:, j, :], tmp, (H + 0.5) ** 2, None, Alu.is_gt
                    )
                # masked multiply on the (static) leading window columns
                nc.vector.tensor_tensor(
                    X[:, :, 0:win], X[:, :, 0:win], keep4, Alu.mult,
                )

                po = potp.tile([NCLS, nfree], f32, tag="po")
                for k in range(KCH):
                    px = pxtp.tile([P, nfree], tr_dt, tag="px")
                    for j in range(super_):
                        nc.tensor.transpose(
                            px[:, j * P:(j + 1) * P],
                            X[:, j, k * P:(k + 1) * P],
                            identr,
                        )
                    xt_t = xtp.tile([P, nfree], mm_dt, tag="xt")
                    if k % 8 < 3:
                        nc.vector.tensor_copy(xt_t, px)
                    else:
                        nc.scalar.copy(xt_t, px)
                    nc.tensor.matmul(
                        po, wt_sb[:, k, :], xt_t,
                        start=(k == 0), stop=(k == KCH - 1),
                    )

                osb = op_.tile([NCLS, nfree], f32, tag="osb")
                nc.vector.tensor_tensor(
                    osb, po, bcol_sb[:, 0:1].to_broadcast((NCLS, nfree)),
                    Alu.add,
                )
                ost = op_.tile([P, super_, NCLS], f32, tag="ost")
                for j in range(super_):
                    pr = ptrp.tile([P, NCLS], f32, tag="pr")
                    nc.tensor.transpose(
                        pr, osb[:, j * P:(j + 1) * P], ident[0:NCLS, 0:NCLS]
                    )
                    nc.vector.tensor_copy(ost[:, j, :], pr)
                nc.sync.dma_start(
                    out_d.ap()[s * nfree:(s + 1) * nfree, :]
                    .rearrange("(j p) n -> p j n", p=P),
                    ost,
                )

    nc.compile()
    return nc


def _get_nc(win, rpc=RPC, super_=4):
    key = (rpc, win, super_)
    if key not in _nc_cache:
        _nc_cache[key] = _build(rpc, win, super_)
    return _nc_cache[key]


def _install_ntff_shim():
    """Provide antenv.axon_hooks (absent in this container) so
    run_bass_kernel_spmd(trace=True) can drive NTFF profiling via the
    axon PJRT .so. Only used for local perf measurement (KERNEL_TRACE=1)."""
    import contextlib
    import ctypes
    import sys as _sys
    import types

    try:
        from antenv.axon_hooks import get_axon_ntff_profile_hook  # noqa: F401
        return
    except ImportError:
        pass

    so_path = "/opt/axon/libaxon_pjrt.so"
    lib = ctypes.CDLL(so_path)
    if not hasattr(lib, "axon_start_nrt_profile"):
        return
    lib.axon_start_nrt_profile.argtypes = [
        ctypes.POINTER(ctypes.c_int64), ctypes.c_size_t,
    ]
    lib.axon_start_nrt_profile.restype = ctypes.c_int64
    lib.axon_stop_nrt_profile.argtypes = [ctypes.c_char_p]
    lib.axon_stop_nrt_profile.restype = ctypes.c_int64

    @contextlib.contextmanager
    def _hook(output_dir, device_ids):
        import jax
        jax.devices()
        if device_ids:
            ids = (ctypes.c_int64 * len(device_ids))(*device_ids)
            rc = lib.axon_start_nrt_profile(ids, len(device_ids))
        else:
            rc = lib.axon_start_nrt_profile(None, 0)
        if rc != 0:
            raise RuntimeError(f"axon_start_nrt_profile rc={rc}")
        try:
            yield
        finally:
            n = lib.axon_stop_nrt_profile(str(output_dir).encode())
            print(f"ntff profile: {n} file(s) -> {output_dir}", file=_sys.stderr)

    mod = types.ModuleType("antenv.axon_hooks")
    mod.get_axon_ntff_profile_hook = lambda: _hook
    mod.set_axon_ntff_profile_hook = lambda h: None
    _sys.modules["antenv.axon_hooks"] = mod


def kernel(x, W, b, starts):
    global last_exec_ns, last_results
    from concourse.bass_utils import run_bass_kernel_spmd

    x = np.ascontiguousarray(np.asarray(x, dtype=np.float32))
    W32 = np.asarray(W, dtype=np.float32)
    b32 = np.asarray(b, dtype=np.float32)
    st = np.asarray(starts, dtype=np.int32).reshape(-1)

    tiles_pc = RPC // P

    perm = np.argsort(st, kind="stable")
    st_s = st[perm]

    # per-core channel rotation: core m handles sorted rows [m*RPC,(m+1)*RPC),
    # whose starts span a narrow range; rotate channels by w_m = min(start) so
    # every row's zero-block lands in the leading `win` rotated columns
    st_c = st_s.reshape(NCORES, RPC)
    w = st_c.min(axis=1).astype(np.int32)          # [NCORES]
    span = (st_c.max(axis=1) - w).astype(np.int64)
    win0 = int(span.max()) + L
    win = min(C, ((win0 + 127) // 128) * 128)
    assert (st_c.max(axis=1) + L <= w + win).all()

    nctr = -(st_s - np.repeat(w, RPC) + H).astype(np.float32)
    WtT = np.ascontiguousarray(W32.T)              # [C, NCLS]
    iota = np.ascontiguousarray(
        np.broadcast_to(np.arange(win, dtype=np.float32), (P, win))
    )
    bcol = np.ascontiguousarray(b32.reshape(NCLS, 1))

    nc = _get_nc(win)

    in_maps = []
    for m in range(NCORES):
        rows = perm[m * RPC:(m + 1) * RPC]
        xs = x[rows]
        wm = int(w[m])
        # rotated weights: rotated channel c <-> original channel (wm+c)%C
        wrot = np.take(WtT, (wm + np.arange(C)) % C, axis=0)
        in_maps.append({
            "xs2": np.concatenate([xs, xs], axis=1),
            "wt": np.ascontiguousarray(
                wrot.reshape(KCH, P, NCLS).transpose(1, 0, 2)
            ),
            "bcol": bcol,
            "iota": iota,
            "nctr": np.ascontiguousarray(
                nctr[m * RPC:(m + 1) * RPC].reshape(tiles_pc, P).T
            ),
            "woff": np.array([[wm]], dtype=np.int32),
        })

    trace = bool(int(os.environ.get("KERNEL_TRACE", "0")))
    if trace:
        _install_ntff_shim()
    res = run_bass_kernel_spmd(
        nc, in_maps, core_ids=list(range(NCORES)), trace=trace
    )
    last_exec_ns = res.exec_time_ns
    last_results = res

    out_s = np.concatenate([r["out"] for r in res.results], axis=0)
    out = np.empty((B, NCLS), dtype=np.float32)
    out[perm] = out_s
    return out
